# revision 12
# baseline (speedup 1.0000x reference)
"""Trainium2 Bass kernel for nn_Alignment.

Per batch b (32 independent blocks):
    a_out = relu(a_in @ W1 + b1)          [512, 768]
    b_out = relu(b_in @ W2 + b2)          [512, 768]
    S     = (a_out @ b_out.T) * temp      [512(s), 512(t)]
    a_att = softmax(S, axis=s);  b_att = softmax(S, axis=t)
    a_feature = a_att.T @ a_in            [512(t), 1536]
    b_feature = b_att @ b_in              [512(s), 1536]

Key structure: both softmaxes share one exp(temp*S); only the
normalizers differ (col-sums for a_att, row-sums for b_att).  The sums
come free via the ScalarE activation accum_out, and each normalizer is
a per-partition scalar folded into the PSUM->SBUF epilogue of the
corresponding feature matmul.  All matmuls run in bf16 (fp32
accumulation in PSUM); transposes are done on the PE as regular matmuls
against a bf16 identity (full-rate, exact).

Sharding: data-parallel over batch -- 4 batches per core on 8 cores,
weights replicated.  Masks are all-ones per the problem spec (mask==1
makes the reference exactly maskless), so they do not enter the device
program.
"""

import functools
from contextlib import ExitStack

import ml_dtypes
import numpy as np

import concourse.tile as tile
from concourse import bacc
from concourse import mybir
from concourse.bass_utils import run_bass_kernel_spmd
from concourse.masks import make_identity

FP32 = mybir.dt.float32
BF16 = mybir.dt.bfloat16
AFT = mybir.ActivationFunctionType

B, L, EH, H = 32, 512, 1536, 768
N_CORES = 8
BPC = B // N_CORES  # batches per core
P = 128
SI = L // P    # 4  seq partition tiles
EJ = EH // P   # 12 embedding partition tiles
HM = H // P    # 6  hidden partition tiles
NF = 512       # matmul free-dim chunk (one PSUM bank of fp32)
NJ = EH // NF  # 3  feature free chunks


def _maybe_loop(tc, repeat: int):
    import contextlib
    if repeat <= 1:
        return contextlib.nullcontext()
    return tc.For_i(0, repeat, 1,
                    hint_engines=(mybir.EngineType.PE, mybir.EngineType.DVE,
                                  mybir.EngineType.Activation, mybir.EngineType.SP))


def _build(temp: float, repeat: int = 1, xbar: bool = True) -> bacc.Bacc:
    nc = bacc.Bacc("TRN2", target_bir_lowering=False)
    a_in = nc.dram_tensor("a_inputs", [BPC, L, EH], FP32, kind="ExternalInput").ap()
    b_in = nc.dram_tensor("b_inputs", [BPC, L, EH], FP32, kind="ExternalInput").ap()
    W1 = nc.dram_tensor("W1bf", [EH, H], BF16, kind="ExternalInput").ap()
    b1 = nc.dram_tensor("b1", [H], FP32, kind="ExternalInput").ap()
    W2 = nc.dram_tensor("W2bf", [EH, H], BF16, kind="ExternalInput").ap()
    b2 = nc.dram_tensor("b2", [H], FP32, kind="ExternalInput").ap()
    a_ft = nc.dram_tensor("a_feature", [BPC, L, EH], FP32, kind="ExternalOutput").ap()
    b_ft = nc.dram_tensor("b_feature", [BPC, L, EH], FP32, kind="ExternalOutput").ap()

    with tile.TileContext(nc) as tc, ExitStack() as ctx:
        consts = ctx.enter_context(tc.tile_pool(name="consts", bufs=1))
        stage = ctx.enter_context(tc.tile_pool(name="stage", bufs=3))
        big = ctx.enter_context(tc.tile_pool(name="big", bufs=2))
        tbuf = ctx.enter_context(tc.tile_pool(name="tbuf", bufs=1))
        proj = ctx.enter_context(tc.tile_pool(name="proj", bufs=1))
        epool = ctx.enter_context(tc.tile_pool(name="epool", bufs=1))
        sums = ctx.enter_context(tc.tile_pool(name="sums", bufs=2))
        outp = ctx.enter_context(tc.tile_pool(name="outp", bufs=4))
        ps1 = ctx.enter_context(tc.tile_pool(name="ps1", bufs=3, space="PSUM"))
        ps3 = ctx.enter_context(tc.tile_pool(name="ps3", bufs=4, space="PSUM"))

        ident = consts.tile([P, P], BF16)
        make_identity(nc, ident)

        def load_cast_side(ib, side, x_dram):
            """DMA one batch side f32 -> SBUF, cast to bf16 natural layout."""
            xb = big.tile([P, SI, EH], BF16, tag=f"{side}_bf")
            for si in range(SI):
                st = stage.tile([P, EH], FP32, tag="stage")
                nc.sync.dma_start(out=st, in_=x_dram[ib, si * P:(si + 1) * P, :])
                nc.vector.tensor_copy(out=xb[:, si, :], in_=st)
            return xb

        # Preload batch 0 before the (bulky) weight DMAs so the PE can
        # start batch-0 transposes while weights stream in.  Only valid
        # for the single-shot program: under the repeat loop the big-pool
        # slots are recycled by later batches, so iterations >1 would
        # read clobbered data.
        preloaded = {}
        if repeat == 1:
            for side, x_dram in (("a", a_in), ("b", b_in)):
                preloaded[(0, side)] = load_cast_side(0, side, x_dram)

        # Weights arrive pre-cast to bf16 (host side); partition-tiled
        # over EH: [P, EJ, H].  Natural layout is already the projection
        # lhsT (contraction on partitions, output-h on the free axis).
        w_bf = []
        for name, w in (("w1", W1), ("w2", W2)):
            wt = consts.tile([P, EJ, H], BF16, name=name, tag=name)
            nc.sync.dma_start(out=wt, in_=w.rearrange("(ko p) h -> p ko h", p=P))
            w_bf.append(wt)
        w1_bf, w2_bf = w_bf

        # Biases [H] -> [P, HM] with bt[p, j] = b[j*P + p] (per-partition
        # bias columns for the projection epilogue).
        b_t = []
        for name, bvec in (("b1t", b1), ("b2t", b2)):
            bt = consts.tile([P, HM], FP32, name=name, tag=name)
            nc.sync.dma_start(out=bt, in_=bvec.rearrange("(j p) -> p j", p=P))
            b_t.append(bt)
        b1_t, b2_t = b_t

        # repeat>1 wraps the whole per-core compute in a hardware
        # loop (timing harness; identical work each iteration).
        with _maybe_loop(tc, repeat):
            for ib in range(BPC):
                # ---- load + cast + transpose inputs ---------------------
                x_bf = {}   # natural [P, SI, EH] bf16 (s on partitions)
                xt_bf = {}  # transposed [P, EJ, L] bf16 (e on partitions)
                for side, x_dram in (("a", a_in), ("b", b_in)):
                    xb = preloaded.pop((ib, side), None)
                    if xb is None:
                        xb = load_cast_side(ib, side, x_dram)
                    xt = tbuf.tile([P, EJ, L], BF16, tag=f"{side}t_bf")
                    # Batch 0 of the single-shot program keeps the PE
                    # identity-matmul transpose so the PE has work while
                    # the weight DMAs stream in; steady-state batches use
                    # the DMA xbar and leave the PE to the real matmuls.
                    if xbar and not (repeat == 1 and ib == 0):
                        # DMA xbar transpose: out[p, ej, s] = in[s, ej*P+p]
                        for si in range(SI):
                            nc.sync.dma_start_transpose(
                                xt[:, :, si * P:(si + 1) * P], xb[:, si, :])
                    else:
                        for ej in range(EJ):
                            pt = ps1.tile([P, L], FP32, tag="ps1")
                            for si in range(SI):
                                nc.tensor.matmul(
                                    pt[:, si * P:(si + 1) * P],
                                    lhsT=xb[:, si, ej * P:(ej + 1) * P],
                                    rhs=ident, start=True, stop=True,
                                )
                            # split PSUM->SBUF evacuation between DVE and ACT
                            if ej % 2 == 0:
                                nc.vector.tensor_copy(out=xt[:, ej, :], in_=pt)
                            else:
                                nc.scalar.activation(out=xt[:, ej, :], in_=pt,
                                                     func=AFT.Copy)
                    x_bf[side] = xb
                    xt_bf[side] = xt

                # ---- projections: outT[h, s] = relu(W.T @ x.T + b) ------
                outT = {}
                for side, wt, bt in (("a", w1_bf, b1_t), ("b", w2_bf, b2_t)):
                    ot = proj.tile([P, HM, L], BF16, tag=f"{side}_outT")
                    xt = xt_bf[side]
                    for hm in range(HM):
                        pt = ps1.tile([P, L], FP32, tag="ps1")
                        for ek in range(EJ):
                            nc.tensor.matmul(
                                pt,
                                lhsT=wt[:, ek, hm * P:(hm + 1) * P],
                                rhs=xt[:, ek, :],
                                start=(ek == 0), stop=(ek == EJ - 1),
                            )
                        nc.scalar.activation(
                            out=ot[:, hm, :], in_=pt,
                            func=AFT.Relu, bias=bt[:, hm:hm + 1],
                        )
                    outT[side] = ot

                # ---- scores + shared exp; row-sums via accum_out --------
                ea = epool.tile([P, SI, L], BF16, tag="ea")        # E[s, t]
                rowsum = sums.tile([P, SI], FP32, tag="rowsum")
                for sm in range(SI):
                    pt = ps1.tile([P, L], FP32, tag="ps1")
                    for hk in range(HM):
                        nc.tensor.matmul(
                            pt,
                            lhsT=outT["a"][:, hk, sm * P:(sm + 1) * P],
                            rhs=outT["b"][:, hk, :],
                            start=(hk == 0), stop=(hk == HM - 1),
                        )
                    nc.scalar.activation(out=ea[:, sm, :], in_=pt,
                                         func=AFT.Exp, scale=temp,
                                         accum_out=rowsum[:, sm:sm + 1])
                rrow = sums.tile([P, SI], FP32, tag="rrow")
                nc.vector.reciprocal(out=rrow, in_=rowsum)

                # ---- transpose E; col-sums via accum_out ----------------
                eat = epool.tile([P, SI, L], BF16, tag="eat")      # E[t, s]
                colsum = sums.tile([P, SI], FP32, tag="colsum")
                for tm in range(SI):
                    pt = ps1.tile([P, L], FP32, tag="ps1")
                    for sk in range(SI):
                        nc.tensor.matmul(
                            pt[:, sk * P:(sk + 1) * P],
                            lhsT=ea[:, sk, tm * P:(tm + 1) * P],
                            rhs=ident, start=True, stop=True,
                        )
                    nc.scalar.activation(out=eat[:, tm, :], in_=pt,
                                         func=AFT.Copy,
                                         accum_out=colsum[:, tm:tm + 1])
                rcol = sums.tile([P, SI], FP32, tag="rcol")
                nc.vector.reciprocal(out=rcol, in_=colsum)

                # ---- a_feature[t, e] = (E.T @ a_nat)[t, e] / colsum[t] --
                for tm in range(SI):
                    pts = [ps3.tile([P, NF], FP32, tag="ps3", name=f"psf{nj}") for nj in range(NJ)]
                    for sk in range(SI):
                        for nj in range(NJ):
                            nc.tensor.matmul(
                                pts[nj],
                                lhsT=ea[:, sk, tm * P:(tm + 1) * P],
                                rhs=x_bf["a"][:, sk, nj * NF:(nj + 1) * NF],
                                start=(sk == 0), stop=(sk == SI - 1),
                            )
                    ot = outp.tile([P, EH], FP32, tag="out")
                    for nj in range(NJ):
                        nc.scalar.activation(out=ot[:, nj * NF:(nj + 1) * NF],
                                             in_=pts[nj], func=AFT.Copy,
                                             scale=rcol[:, tm:tm + 1])
                    nc.sync.dma_start(out=a_ft[ib, tm * P:(tm + 1) * P, :], in_=ot)

                # ---- b_feature[s, e] = (E @ b_nat)[s, e] / rowsum[s] ----
                for sm in range(SI):
                    pts = [ps3.tile([P, NF], FP32, tag="ps3", name=f"psf{nj}") for nj in range(NJ)]
                    for tk in range(SI):
                        for nj in range(NJ):
                            nc.tensor.matmul(
                                pts[nj],
                                lhsT=eat[:, tk, sm * P:(sm + 1) * P],
                                rhs=x_bf["b"][:, tk, nj * NF:(nj + 1) * NF],
                                start=(tk == 0), stop=(tk == SI - 1),
                            )
                    ot = outp.tile([P, EH], FP32, tag="out")
                    for nj in range(NJ):
                        nc.vector.tensor_scalar_mul(
                            out=ot[:, nj * NF:(nj + 1) * NF],
                            in0=pts[nj], scalar1=rrow[:, sm:sm + 1])
                    nc.sync.dma_start(out=b_ft[ib, sm * P:(sm + 1) * P, :], in_=ot)

    nc.compile()
    return nc


@functools.lru_cache(maxsize=4)
def _build_cached(temp: float, repeat: int = 1, xbar: bool = True) -> bacc.Bacc:
    return _build(temp, repeat, xbar)


def _run(inputs: dict, trace: bool = False):
    a_inputs = np.ascontiguousarray(np.asarray(inputs["a_inputs"], dtype=np.float32))
    b_inputs = np.ascontiguousarray(np.asarray(inputs["b_inputs"], dtype=np.float32))
    W1bf = np.ascontiguousarray(
        np.asarray(inputs["W1"], dtype=np.float32).astype(ml_dtypes.bfloat16))
    b1 = np.ascontiguousarray(np.asarray(inputs["b1"], dtype=np.float32))
    W2bf = np.ascontiguousarray(
        np.asarray(inputs["W2"], dtype=np.float32).astype(ml_dtypes.bfloat16))
    b2 = np.ascontiguousarray(np.asarray(inputs["b2"], dtype=np.float32))
    temp = float(np.asarray(inputs["temperature"]))

    nc = _build_cached(temp)
    in_maps = []
    for c in range(N_CORES):
        sl = slice(c * BPC, (c + 1) * BPC)
        in_maps.append({
            "a_inputs": a_inputs[sl],
            "b_inputs": b_inputs[sl],
            "W1bf": W1bf, "b1": b1, "W2bf": W2bf, "b2": b2,
        })
    res = run_bass_kernel_spmd(nc, in_maps, list(range(N_CORES)), trace=trace)
    a_feat = np.concatenate([res.results[c]["a_feature"] for c in range(N_CORES)], axis=0)
    b_feat = np.concatenate([res.results[c]["b_feature"] for c in range(N_CORES)], axis=0)
    return (a_feat, b_feat), res


def kernel(a_inputs, a_mask, b_inputs, b_mask, W1, b1, W2, b2, temperature):
    (a_feat, b_feat), _ = _run({
        "a_inputs": a_inputs, "b_inputs": b_inputs,
        "W1": W1, "b1": b1, "W2": W2, "b2": b2,
        "temperature": temperature,
    })
    return (a_feat, b_feat)
